# revision 1
# baseline (speedup 1.0000x reference)
"""Multi-head causal attention with RoPE (B=1, S=4096, D=1024, H=16) on 8
Trainium2 NeuronCores.

Sharding: tensor-parallel over heads - each core computes 2 heads (QKV
projections column-sliced, attention, and its rank-128 partial of the output
projection; host sums the 8 partials = row-parallel wo).

Design (v3):
  - QKV projections in hi/lo fp8 DoubleRow form: x = xh(e4m3)+xl(e5m2) and
    w*32 = wh(e4m3)+wl(e5m2) host-side; (wh.xh + wl.xh + wh.xl) via three
    DoubleRow groups (K=256/instr, 0.5 cyc/row) gives ~bf16 accuracy at 75%
    of the f32r PE cost. v is computed already transposed ([seq, head_dim])
    by swapping matmul operands, so no PE transposes are needed for vext.
  - RoPE without swap-projections: ACT copies the projection PSUM to fp16,
    a partition-shift SBUF DMA builds the pair-partner tensor, sign and all
    scale factors folded into host-prepped cos/sin rows.
  - scores / attnV in fp16 at 1 cyc/row; above-diagonal 128x512 tiles are
    skipped and diagonal tiles are narrowed to their valid query range.
  - softmax exp via the Schraudolph bit-trick: cos/sin carry
    sqrt(1024*log2(e)/8)/32 so score PSUM holds y = z*1477.32 where z is the
    true logit; ex_fp16bits = int16(min(y+15360, gate)) is ONE tensor_scalar
    (or scalar_tensor_tensor with a 0/31743 gate on diagonal tiles, folding
    the causal mask: masked lanes become +0.0 exactly). A weighted share of
    tiles runs native exp on ACT (scale=1/1477.32, bias=ln(1.0407) matching
    the trick's mean ratio); the rest run the trick on DVE.
  - denominators from a ones(=32, cancelling the fp8 weight rescale) column
    appended to vext; normalization via a PE-broadcast reciprocal row and
    DVE muls; o-proj is a single K=128 fp16 matmul per 128-dout tile (the
    h1 half moves into afin partitions 64:128 by a small SBUF DMA).
  - scheduling: per-chunk epilogue (broadcast+normalize+o-proj) is deferred
    past the NEXT chunk's projections; reciprocal+partition-move DMAs issue
    early; exp ops and PSUM-evacuation copies are round-robined over ACT and
    DVE (GPSIMD cannot touch PSUM); next-chunk x/cos DMAs prefetch behind
    the latency-critical shift DMAs; the last chunk writes its output in
    per-tile DMAs to shorten the drain.
Hardware-validated: rel_err ~8.9e-3 vs the f32 reference (gate 2e-2).
"""
import math
import numpy as np

import concourse.bass as bass
import concourse.mybir as mybir
import concourse.tile as tile
from concourse.bass_utils import run_bass_kernel_spmd
from concourse.alu_op_type import AluOpType

B, S, D, H = 1, 4096, 1024, 16
HD = D // H            # 64
NC = 8                 # cores
HPC = H // NC          # 2 heads per core
SQC = 512              # seq chunk (matmul free dim)
NJ = S // SQC          # 8 chunks
NKT = S // 128         # 32 sk partition tiles
KT = D // 128          # 8 contraction tiles for projections

F32 = mybir.dt.float32
F32R = mybir.dt.float32r
F16 = mybir.dt.float16
BF16 = mybir.dt.bfloat16
I16 = mybir.dt.int16
F8E4 = mybir.dt.float8e4
F8E5 = mybir.dt.float8e5
DRMODE = mybir.MatmulPerfMode.DoubleRow
AF = mybir.ActivationFunctionType

L2E1024 = math.log2(math.e) * 1024.0          # 1477.3197
CS = math.sqrt(L2E1024 / 8.0)                 # 13.5891 folded into cos/sin
EBIAS = 15360.0                               # fp16 exponent bias << 10
ECLAMP = 31743.0                              # just below fp16 +inf bits
ACT_SCALE = 1.0 / L2E1024
ACT_BIAS = math.log(1.0406936)                # match trick's mean ratio

_MAX_WAITS = 1


def _fix_waits(nc):
    """walrus in this container rejects >1 sync-wait per instruction
    ("Too many sync wait commands"); split excess waits onto preceding
    same-engine NoOps (engine blocks in order, semantics preserved)."""
    n = 0
    for fn in nc.m.functions:
        for bb in fn.blocks:
            new_list = []
            for inst in bb.instructions:
                si = getattr(inst, "sync_info", None)
                if si is not None and si.on_wait and len(si.on_wait) > _MAX_WAITS:
                    waits = list(si.on_wait)
                    excess, keep = waits[:-_MAX_WAITS], waits[-_MAX_WAITS:]
                    for j in range(0, len(excess), _MAX_WAITS):
                        nop = mybir.InstNoOp(
                            name=f"I-waitfix-{nc.next_id()}",
                            ins=[],
                            outs=[],
                            engine=inst.engine,
                            sync_info=mybir.SyncInfo(
                                on_wait=excess[j : j + _MAX_WAITS], on_update=[]
                            ),
                        )
                        nc.register_instruction(nop)
                        new_list.append(nop)
                        n += 1
                    si.on_wait = keep
                new_list.append(inst)
            bb.instructions[:] = new_list
    return n


def build_program(mode: str, opts=None):
    """mode: 'causal' (skip above-diag tiles; gate tiles fold the mask),
    'zeros' (no mask, full attention), 'general' (additive mask, full)."""
    causal = mode == "causal"
    o = {
        # exp engine shares out of 8 (non-diag): ACT/DVE/POOL
        "exp_w": (3, 1, 0),
        # diag-pair engine shares (DVE/POOL)
        "diag_w": (2, 2),
        # out-copy engine cycle
        "cp_w": (1, 2, 0),   # DVE/ACT/POOL (pool cannot read PSUM)
        "ex_bufs": 14,
        "sc_bufs": 3,
        "ps_bufs": 3,
        "swpipe": 4,
        "pf_late": True,
        "af_pool": False,
        "rt_pool": False,
        "ep_sc": False,
        "vx_act": False,
        "ep_split": False,
        "v_ring": False,
        "vx_alt": False,
        "ep_mid": False,
        "sbuf_bufs": 3,
        "kadd_pool": False,
        "qadd_pool": False,
        "out_defer": 0,
        "oo_bufs": 6,
        "op_attn": False,
        "early_split": 0,
    }
    if opts:
        o.update(opts)
    nc = bass.Bass()

    xh_d = nc.dram_tensor("xh", (D, S), F8E4, kind="ExternalInput")
    xl_d = nc.dram_tensor("xl", (D, S), F8E5, kind="ExternalInput")
    w_d = {}
    for nm in ("wqh", "wkh", "wvh"):
        w_d[nm] = nc.dram_tensor(nm, (128, KT * 128), F8E4, kind="ExternalInput")
    for nm in ("wql", "wkl", "wvl"):
        w_d[nm] = nc.dram_tensor(nm, (128, KT * 128), F8E5, kind="ExternalInput")
    wo_d = nc.dram_tensor("wo", (128, D), F16, kind="ExternalInput")
    cossin_d = nc.dram_tensor("cossin", (128, 2 * S), F16, kind="ExternalInput")
    if causal:
        gd_d = nc.dram_tensor("gdiag", (128, 4 * SQC), F32, kind="ExternalInput")
    elif mode == "general":
        mask_d = nc.dram_tensor("maskT", (NJ, S, SQC), F32, kind="ExternalInput")
    out_d = nc.dram_tensor("opT", (D, S), BF16, kind="ExternalOutput")

    # global engine round-robin state
    cnt = {"exp": 0, "diag": 0, "cp": 0}

    def pick(kind, weights, engines):
        tot = sum(weights)
        i = cnt[kind] % tot
        cnt[kind] += 1
        acc = 0
        for w, e in zip(weights, engines):
            acc += w
            if i < acc:
                return e
        return engines[-1]

    with tile.TileContext(nc) as tc:
        with (
            tc.tile_pool(name="wts", bufs=1) as wts,
            tc.tile_pool(name="big", bufs=1) as big,
            tc.tile_pool(name="xc", bufs=2) as xcp,
            tc.tile_pool(name="cs", bufs=2) as csp,
            tc.tile_pool(name="qs", bufs=o["sbuf_bufs"]) as qsp,
            tc.tile_pool(name="rt", bufs=o["sbuf_bufs"]) as rtp,
            tc.tile_pool(name="qr", bufs=o["sbuf_bufs"]) as qrp,
            tc.tile_pool(name="ex", bufs=o["ex_bufs"]) as exp_p,
            tc.tile_pool(name="mk", bufs=3) as mkp,
            tc.tile_pool(name="af", bufs=o["sbuf_bufs"]) as afp,
            tc.tile_pool(name="rc", bufs=o["sbuf_bufs"]) as rcp,
            tc.tile_pool(name="oo", bufs=o["oo_bufs"]) as oop,
            tc.tile_pool(name="pp", bufs=o["ps_bufs"], space=bass.MemorySpace.PSUM) as ppp,
            tc.tile_pool(name="sc", bufs=o["sc_bufs"], space=bass.MemorySpace.PSUM) as scp_p,
            tc.tile_pool(name="at0", bufs=1, space=bass.MemorySpace.PSUM) as at0p,
            tc.tile_pool(name="at1", bufs=1, space=bass.MemorySpace.PSUM) as at1p,
        ):
            # ---- q weights + chunk-0 inputs first so q-proj(0) starts ASAP
            w_sb = {}
            for nm in ("wqh", "wql"):
                dt8 = F8E4 if nm.endswith("h") else F8E5
                w_sb[nm] = wts.tile([128, KT, 128], dt8, tag=nm, name=nm)
                nc.sync.dma_start(
                    w_sb[nm][:], w_d[nm].rearrange("p (k m) -> p k m", k=KT)
                )
            # split per k-pair: the first DR matmul only needs k-tiles 0:2
            xc0 = xcp.tile([128, KT, SQC], F8E4, tag="xc", name="xc")
            for k0 in range(0, KT, 2):
                nc.sync.dma_start(
                    xc0[:, k0 : k0 + 2, :],
                    xh_d[k0 * 128 : (k0 + 2) * 128, 0:SQC].rearrange(
                        "(k p) n -> p k n", p=128
                    ),
                )
            xcl0 = xcp.tile([128, KT, SQC], F8E5, tag="xcl", name="xcl")
            nc.sync.dma_start(
                xcl0[:], xl_d[:, 0:SQC].rearrange("(k p) n -> p k n", p=128)
            )
            csl0 = csp.tile([128, 2, SQC], F16, tag="cs", name="cs")
            nc.sync.dma_start(
                csl0[:], cossin_d.rearrange("p (c s) -> p c s", c=2)[:, :, 0:SQC]
            )
            # ---- weights / constants ----
            for nm in ("wkh", "wvh", "wkl", "wvl"):
                dt8 = F8E4 if nm.endswith("h") else F8E5
                w_sb[nm] = wts.tile([128, KT, 128], dt8, tag=nm, name=nm)
                nc.sync.dma_start(
                    w_sb[nm][:], w_d[nm].rearrange("p (k m) -> p k m", k=KT)
                )
            wo_sb = wts.tile([128, D], F16, tag="wo", name="wo_sb")
            nc.sync.dma_start(wo_sb[:], wo_d[:])
            gd_sb = None
            if causal:
                gd_sb = wts.tile([128, 4, SQC], F32, tag="gd", name="gd_sb")
                nc.sync.dma_start(gd_sb[:], gd_d.rearrange("p (r n) -> p r n", r=4))

            ones1 = wts.tile([1, 64], F32R, tag="ones1", name="ones1")
            onesf = wts.tile([1, 64], F32, tag="onesf", name="onesf")
            nc.vector.memset(onesf[:], 1.0)
            nc.vector.tensor_copy(ones1[:], onesf[:])
            biast = wts.tile([128, 1], F32, tag="biast", name="biast")
            nc.vector.memset(biast[:], ACT_BIAS)

            krot = big.tile([128, S], F16, tag="krot", name="krot")
            vext = big.tile([128, NKT, 2, 65], F16, tag="vext", name="vext")
            onesv = wts.tile([128, NKT * 2], F32, tag="onesv", name="onesv")
            nc.vector.memset(onesv[:], 32.0)
            nc.vector.tensor_copy(
                vext[:].rearrange("p i h c -> p (i h) c")[:, :, 64], onesv[:]
            )

            def proj(wname, xc, xcl, ps):
                # hi/lo fp8 DoubleRow: (wh+wl)(xh+xl) ~ wh.xh + wl.xh + wh.xl
                wh = w_sb[wname + "h"]
                wl = w_sb[wname + "l"]
                nmm = 0
                for grp_l, grp_r in ((wh, xc), (wl, xc), (wh, xcl)):
                    for k in range(0, KT, 2):
                        nmm += 1
                        nc.tensor.matmul(
                            ps[:], grp_l[:, k : k + 2, :], grp_r[:, k : k + 2, :],
                            start=(nmm == 1), stop=(nmm == 12),
                            perf_mode=DRMODE,
                        )

            def rot_path(ps, csl, dst_ap, tag):
                # dst = ps*cos + shift(ps*sin'') where sin''[p] = sin'[p^1]
                # (host-prepped): multiplying BEFORE the partition-shift DMA
                # removes the ACT staging copy from the critical chain
                t2 = qsp.tile([128, SQC], F16, tag=f"{tag}16", name=f"{tag}t2")
                nc.vector.tensor_tensor(t2[:], ps[:], csl[:, 1, :], op=AluOpType.mult)
                tsh = qsp.tile([128, SQC], F16, tag=f"{tag}sh", name=f"{tag}sh")
                nc.sync.dma_start(tsh[0:127:2, :], t2[1:128:2, :])
                nc.sync.dma_start(tsh[1:128:2, :], t2[0:127:2, :])
                t1 = rtp.tile([128, SQC], F16, tag=f"{tag}t1", name=f"{tag}t1")
                nc.vector.tensor_tensor(t1[:], ps[:], csl[:, 0, :], op=AluOpType.mult)
                # the final add is SBUF-only: k's add can run on the otherwise
                # idle Pool engine (k-rot is only needed by late diag tiles)
                if tag == "q" and o["qadd_pool"]:
                    nc.gpsimd.tensor_add(dst_ap, t1[:], tsh[:])
                elif tag == "k" and o["kadd_pool"]:
                    nc.gpsimd.tensor_add(dst_ap, t1[:], tsh[:])
                else:
                    reng = nc.gpsimd if o["rt_pool"] else nc.vector
                    reng.tensor_add(dst_ap, t1[:], tsh[:])
                return t1, tsh

            def load_chunk(j):
                sl = slice(j * SQC, (j + 1) * SQC)
                xc = xcp.tile([128, KT, SQC], F8E4, tag="xc", name="xc")
                nc.sync.dma_start(xc[:], xh_d[:, sl].rearrange("(k p) n -> p k n", p=128))
                xcl = xcp.tile([128, KT, SQC], F8E5, tag="xcl", name="xcl")
                nc.sync.dma_start(xcl[:], xl_d[:, sl].rearrange("(k p) n -> p k n", p=128))
                csl = csp.tile([128, 2, SQC], F16, tag="cs", name="cs")
                nc.sync.dma_start(
                    csl[:], cossin_d.rearrange("p (c s) -> p c s", c=2)[:, :, sl]
                )
                return xc, xcl, csl

            def do_proj_chunk(j, loaded, nxt, pend=None):
                sl = slice(j * SQC, (j + 1) * SQC)
                xc, xcl, csl = loaded
                # q first: its rot chain gates all of this chunk's scores
                ps_q = ppp.tile([128, SQC], F32, tag="ps", name="ps_q")
                proj("wq", xc, xcl, ps_q)
                afin_prev = None
                if pend is not None and o["ep_split"]:
                    afin_prev = pend[0]()
                qrot = qrp.tile([128, SQC], F16, tag="qrot", name="qrot")
                q_t1, q_tsh = rot_path(ps_q, csl, qrot[:], "q")
                nxt_loaded = None
                if not o["pf_late"]:
                    nxt_loaded = load_chunk(nxt) if nxt is not None else None
                # v (transposed): out [sk, hd] per 128-sk tile
                ps_v = ppp.tile([128, SQC], F32, tag="ps", name="ps_v")
                for r in range(4):
                    rsl = slice(r * 128, (r + 1) * 128)
                    nmm = 0
                    for grp_l, grp_r in (
                        (xc, w_sb["wvh"]), (xc, w_sb["wvl"]), (xcl, w_sb["wvh"])
                    ):
                        for k in range(0, KT, 2):
                            nmm += 1
                            nc.tensor.matmul(
                                ps_v[:, rsl],
                                grp_l[:, k : k + 2, rsl],
                                grp_r[:, k : k + 2, :],
                                start=(nmm == 1), stop=(nmm == 12),
                                skip_group_check=True,
                                perf_mode=DRMODE,
                            )
                    if o["vx_act"]:
                        nc.scalar.copy(vext[:, 4 * j + r, :, 0:64], ps_v[:, rsl])
                    else:
                        nc.vector.tensor_copy(
                            vext[:, 4 * j + r, :, 0:64], ps_v[:, rsl]
                        )
                # previous chunk's epilogue here (knob): its PE work lands
                # after q+v proj, and its DVE muls run before k's rot ops
                if pend is not None and o["ep_mid"]:
                    pend()
                # k last: krot(j) is only needed by this chunk's diagonal
                # tiles, which come at the end of the i loop
                ps_k = ppp.tile([128, SQC], F32, tag="ps", name="ps_k")
                proj("wk", xc, xcl, ps_k)
                rot_path(ps_k, csl, krot[:, sl], "k")
                if pend is not None and o["ep_split"]:
                    pend[1](afin_prev)
                # prefetch next chunk's inputs AFTER the latency-critical
                # q/k shift DMAs so they are not stuck behind a big transfer
                if o["pf_late"]:
                    nxt_loaded = load_chunk(nxt) if nxt is not None else None
                return (qrot, q_t1, q_tsh), nxt_loaded

            def exp_one(scp_s, ex_s, exb_s, eng, gate):
                """exp of one [128, SQC] head-slice on the given engine."""
                if eng is nc.scalar:
                    nc.scalar.activation(
                        ex_s, scp_s, AF.Exp, bias=biast[:], scale=ACT_SCALE
                    )
                elif gate is not None:
                    eng.scalar_tensor_tensor(
                        exb_s, scp_s, EBIAS, gate, AluOpType.add, AluOpType.min
                    )
                else:
                    eng.tensor_scalar(
                        exb_s, scp_s, EBIAS, ECLAMP, AluOpType.add, AluOpType.min
                    )

            def emit_exp(scp, ex, i, j, qlo=0):
                """one elementwise op: ex(f16 bits) = min(y+EBIAS, gate) int16"""
                exb = ex[:, qlo:].bitcast(I16)
                diag = causal and i >= 4 * j
                if causal or mode == "zeros":
                    if diag:
                        g = gd_sb[:, i - 4 * j, qlo:]
                        exp_one(scp[:, qlo:], ex[:, qlo:], exb, nc.vector, g)
                    else:
                        eng = pick(
                            "exp", o["exp_w"], (nc.scalar, nc.vector, nc.gpsimd)
                        )
                        exp_one(scp[:, qlo:], ex[:, qlo:], exb, eng, None)
                else:
                    mt = mkp.tile([128, SQC], F32, tag="mk", name="mt")
                    nc.sync.dma_start(mt[:], mask_d[j, i * 128 : (i + 1) * 128, :])
                    t = mkp.tile([128, SQC], F32, tag="mks", name="mts")
                    nc.vector.tensor_add(t[:], scp[:], mt[:])
                    t2 = mkp.tile([128, SQC], F32, tag="mks2", name="mts2")
                    nc.vector.tensor_scalar(
                        t2[:], t[:], EBIAS, ECLAMP, AluOpType.add, AluOpType.min
                    )
                    nc.vector.tensor_scalar_max(exb, t2[:], 0.0)

            def do_attn_chunk(j, qrot_parts, oproj_after=None):
                qrot, q_t1, q_tsh = qrot_parts
                sl = slice(j * SQC, (j + 1) * SQC)
                nkt_j = 4 * (j + 1) if causal else NKT
                at = [
                    at0p.tile([65, SQC], F32, tag="at0", name="at0"),
                    at1p.tile([65, SQC], F32, tag="at1", name="at1"),
                ]
                pend = []

                def emit_scores(i):
                    # diagonal tiles only have valid queries n >= (i-4j)*128
                    qlo = (i - 4 * j) * 128 if (causal and i > 4 * j) else 0
                    exs = []
                    for h in range(HPC):
                        hsl = slice(h * 64, (h + 1) * 64)
                        scp = scp_p.tile([128, SQC], F32, tag="scp", name="scp")
                        if i < o["early_split"]:
                            # qrot = t1 + shift(t2) is not summed yet for the
                            # first tiles; k.t1 has no DMA in its dep chain so
                            # PE fills the chunk-start window, and k.tsh
                            # accumulates once the shift DMA lands
                            nc.tensor.matmul(
                                scp[:, qlo:],
                                krot[hsl, i * 128 : (i + 1) * 128],
                                q_t1[hsl, qlo:],
                                start=True, stop=False,
                            )
                            nc.tensor.matmul(
                                scp[:, qlo:],
                                krot[hsl, i * 128 : (i + 1) * 128],
                                q_tsh[hsl, qlo:],
                                start=False, stop=True,
                            )
                        else:
                            nc.tensor.matmul(
                                scp[:, qlo:],
                                krot[hsl, i * 128 : (i + 1) * 128],
                                qrot[hsl, qlo:],
                                start=True, stop=True,
                            )
                        ex = exp_p.tile([128, SQC], F16, tag="ex", name="ex")
                        emit_exp(scp, ex, i, j, qlo)
                        exs.append(ex)
                    return exs, qlo

                def emit_attnv(i, exs, qlo):
                    for h in range(HPC):
                        nc.tensor.matmul(
                            at[h][:, qlo:],
                            vext[:, i, h, :],
                            exs[h][:, qlo:],
                            start=(i == 0), stop=(i == nkt_j - 1),
                        )

                depth = o["swpipe"]
                for i in range(nkt_j):
                    exs, qlo = emit_scores(i)
                    pend.append((i, exs, qlo))
                    if i == min(1, nkt_j - 1) and oproj_after is not None:
                        # previous chunk's o-proj lands here: the first score
                        # tiles keep PE busy while its afin chain completes
                        oproj_after()
                    if len(pend) > depth:
                        ii, exx, ql = pend.pop(0)
                        emit_attnv(ii, exx, ql)
                for ii, exx, ql in pend:
                    emit_attnv(ii, exx, ql)

                # reciprocals + rec0 partition-move DMAs immediately: they
                # sit in the queues ahead of the next chunk's big transfers.
                # at[h] rows are staged to SBUF (f32: unnormalized values can
                # exceed fp16 range) in parallel with the reciprocal chain so
                # the afin mul can read bcs directly from PSUM later.
                recs = []
                at_sb = []
                for h in range(HPC):
                    rec = rcp.tile([65, SQC], F32R, tag="rec", name="rec")
                    with nc.allow_low_precision("f32r reciprocal of softmax denom"):
                        nc.vector.reciprocal(rec[64:65, :], at[h][64:65, :])
                    rec0 = rcp.tile([1, SQC], F32R, tag="rec0", name="rec0")
                    nc.sync.dma_start(rec0[:], rec[64:65, :])
                    recs.append(rec0[:])
                    asb = afp.tile([64, SQC], F32, tag="atsb", name="at_sb")
                    nc.scalar.copy(asb[:], at[h][0:64, :])
                    at_sb.append(asb)

                def norm_part():
                    # broadcast 1/denom across partitions via a rank-1 matmul;
                    # deferred past the next chunk's q-proj so the reciprocal
                    # chain is already done. h1 first so its afin move DMA
                    # overlaps h0's work.
                    afin = afp.tile([128, SQC], F16, tag="afin", name="afin")
                    for h in (1, 0):
                        bpool = scp_p if o["ep_sc"] else ppp
                        btag = "scp" if o["ep_sc"] else "ps"
                        bcs = bpool.tile([128, SQC], F32, tag=btag, name="bcs")
                        nc.tensor.matmul(
                            bcs[0:64, :], ones1[:], recs[h],
                            start=True, stop=True,
                        )
                        # at rows were pre-staged to SBUF, so the mul may
                        # read the broadcast directly from PSUM (only one
                        # PSUM operand per tensor_tensor is allowed)
                        if h == 0:
                            nc.vector.tensor_tensor(
                                afin[0:64, :], at_sb[0][:], bcs[0:64, :],
                                op=AluOpType.mult,
                            )
                        else:
                            tmph = afp.tile([64, SQC], F16, tag="tmph", name="tmph")
                            nc.vector.tensor_tensor(
                                tmph[:], at_sb[1][:], bcs[0:64, :],
                                op=AluOpType.mult,
                            )
                            nc.sync.dma_start(afin[64:128, :], tmph[:])
                    return afin

                def oproj_part(afin):
                    os_big = oop.tile([128, KT, SQC], BF16, tag="oo", name="os_big")
                    opool = scp_p if o["ep_sc"] else ppp
                    otag = "scp" if o["ep_sc"] else "ps"
                    for dt_i in range(KT):
                        opt_t = opool.tile([128, SQC], F32, tag=otag, name="op")
                        op = opt_t[:]
                        nc.tensor.matmul(
                            op,
                            wo_sb[:, dt_i * 128 : (dt_i + 1) * 128],
                            afin[:],
                            start=True, stop=True,
                        )
                        if j == NJ - 1:
                            # drain: strict DVE/ACT alternation maximizes
                            # copy parallelism with nothing else running
                            eng = nc.vector if dt_i % 2 == 0 else nc.scalar
                        else:
                            eng = pick(
                                "cp", o["cp_w"], (nc.vector, nc.scalar, nc.gpsimd)
                            )
                        if eng is nc.scalar:
                            nc.scalar.copy(os_big[:, dt_i, :], op)
                        else:
                            eng.tensor_copy(os_big[:, dt_i, :], op)
                    if j == NJ - 1:
                        for dt_i in range(KT):
                            nc.sync.dma_start(
                                out_d[dt_i * 128 : (dt_i + 1) * 128, sl],
                                os_big[:, dt_i, :],
                            )
                    elif j < o["out_defer"]:
                        # early chunks: DMA device is saturated by x loads
                        # while the pipeline fills; park the output tile and
                        # write it back during the DMA-idle late phase
                        pend_out.append((os_big, sl))
                    else:
                        nc.sync.dma_start(
                            out_d[:, sl].rearrange("(k p) n -> p k n", p=128),
                            os_big[:],
                        )
                        if pend_out:
                            ob, osl = pend_out.pop(0)
                            nc.sync.dma_start(
                                out_d[:, osl].rearrange("(k p) n -> p k n", p=128),
                                ob[:],
                            )

                return (norm_part, oproj_part)

            pend_oproj = None
            pend_out = []
            loaded = (xc0, xcl0, csl0)
            for j in range(NJ):
                nxt = j + 1 if j + 1 < NJ else None
                qrot, loaded = do_proj_chunk(j, loaded, nxt, pend_oproj)
                hook = None
                if pend_oproj is not None:
                    if o["op_attn"]:
                        np_, op_ = pend_oproj
                        afin_prev = np_()
                        hook = (lambda op=op_, af=afin_prev: op(af))
                    else:
                        pend_oproj[1](pend_oproj[0]())
                pend_oproj = do_attn_chunk(j, qrot, oproj_after=hook)
            pend_oproj[1](pend_oproj[0]())

    _fix_waits(nc)
    return nc


def _host_prep(x, cos, sin, mask, wq, wk, wv, wo):
    x = np.asarray(x, dtype=np.float32)
    cos = np.asarray(cos, dtype=np.float32)
    sin = np.asarray(sin, dtype=np.float32)
    mask = np.asarray(mask, dtype=np.float32)
    wq = np.asarray(wq, dtype=np.float32)
    wk = np.asarray(wk, dtype=np.float32)
    wv = np.asarray(wv, dtype=np.float32)
    wo = np.asarray(wo, dtype=np.float32)
    import ml_dtypes

    E4 = ml_dtypes.float8_e4m3
    E5 = ml_dtypes.float8_e5m2
    xT = np.ascontiguousarray(x.reshape(S, D).T)
    xh = xT.astype(E4)
    xl = (xT - xh.astype(np.float32)).astype(E5)

    # cos/sin rows: partition p -> rotation pair (p % 64)//2; sign on sin
    idx = np.repeat(np.arange(HD // 2), 2)                 # (64,)
    cosr = cos[:, idx].T                                   # (64, S)
    sinr = sin[:, idx].T
    sgn = np.where(np.arange(HD) % 2 == 0, -1.0, 1.0)[:, None]
    csc = CS / 32.0                       # undo the 32x fp8 weight rescale
    cos128 = np.vstack([cosr, cosr]) * csc
    sin128 = np.vstack([sinr * sgn, sinr * sgn]) * csc
    # sin''[p] = sin'[p^1]: the mul happens BEFORE the partition shift
    sin128 = sin128.reshape(64, 2, S)[:, ::-1, :].reshape(128, S)
    cossin = np.concatenate(
        [cos128[:, None, :], sin128[:, None, :]], axis=1
    ).reshape(128, 2 * S).astype(np.float16)

    neg = np.isneginf(mask)
    triu = np.triu(np.ones((S, S), dtype=bool), 1)
    if not neg.any() and not mask.any():
        mode = "zeros"
    elif np.array_equal(neg, triu) and not mask[~neg].any():
        mode = "causal"
        blk0 = mask[0:SQC, 0:SQC]
        # gate[p, r, n] over score tile [sk=128, sq=512]: valid iff not -inf
        gd = np.empty((128, 4, SQC), np.float32)
        for r in range(4):
            blkv = ~np.isneginf(blk0[:, r * 128 : (r + 1) * 128])  # (q, k)
            gd[:, r, :] = np.where(blkv.T, ECLAMP, 0.0)
        gdiag = np.ascontiguousarray(gd.reshape(128, 4 * SQC))
    else:
        mode = "general"
        maskT = np.empty((NJ, S, SQC), np.float32)
        for j in range(NJ):
            maskT[j] = mask[j * SQC : (j + 1) * SQC, :].T * np.float32(L2E1024)

    per_core = []
    for c in range(NC):
        hs, he = c * 128, (c + 1) * 128
        m = {"xh": xh, "xl": xl, "cossin": cossin}
        # lhsT weight layout: w_t[p, k*128+m] = 32 * w_slice[m, k*128+p]
        for name, w in (("wq", wq), ("wk", wk), ("wv", wv)):
            ws = w[hs:he, :] * np.float32(32.0)            # (128, D)
            t = ws.T.reshape(KT, 128, 128)                 # (k, p, m)
            wt = np.ascontiguousarray(t.transpose(1, 0, 2).reshape(128, D))
            whq = wt.astype(E4)
            m[name + "h"] = whq
            m[name + "l"] = (wt - whq.astype(np.float32)).astype(E5)
        m["wo"] = np.ascontiguousarray(wo[:, hs:he].T).astype(np.float16)
        if mode == "causal":
            m["gdiag"] = gdiag
        elif mode == "general":
            m["maskT"] = maskT
        per_core.append(m)
    return mode, per_core


_cache = {}


def kernel(x, cos, sin, mask, wq, wk, wv, wo, start_pos=0, **_):
    mode, in_maps = _host_prep(x, cos, sin, mask, wq, wk, wv, wo)
    if mode not in _cache:
        _cache[mode] = build_program(mode)
    nc = _cache[mode]
    res = run_bass_kernel_spmd(nc, in_maps, core_ids=list(range(NC)))
    acc = np.zeros((D, S), np.float64)
    for c in range(NC):
        acc += res.results[c]["opT"].astype(np.float64)
    return np.ascontiguousarray(acc.T).reshape(B, S, D).astype(np.float32)

